# revision 1
# baseline (speedup 1.0000x reference)
"""Trainium2 Bass kernel v2 for GQA attention (nn_Attention_74302934220843).

Tensor-parallel over heads (2 q-heads + 1 kv-head per core), AllToAll on the
attention output, token-sharded wo. v2 vs baseline:
 - Q resident in SBUF f32r (no DRAM roundtrip between projection/attention).
 - K/V/weights/ao stored bf16 (stationary operands - no LDW split, cost keys
   on the moving operand which stays f32r everywhere).
 - rope via stacked [cos;cos] / [-sin;sin] tables: 5 full-width DVE ops.
 - rms-norm rsqrt = exp(-0.5*ln(x)) on ScalarE - single act-table set for
   the whole kernel (exp/ln/square/copy share one set; sqrt does not).
 - softmax denominator on PE (ones-matmul PSUM accumulation, exact fp32)
   or DVE bf16 packed adds, switchable via DEN_PE.
 - a2a payload bf16 (half the wire bytes).
"""
import sys

for _p in ("/opt/trn_rl_repo", "/root/.axon_site/_ro/trn_rl_repo"):
    if _p not in sys.path:
        sys.path.insert(0, _p)

import numpy as np
import ml_dtypes
import concourse.bass as bass
import concourse.mybir as mybir
import concourse.tile as tile
from concourse import bacc
from concourse.bass_utils import run_bass_kernel_spmd

F32 = mybir.dt.float32
F32R = mybir.dt.float32r
BF16 = mybir.dt.bfloat16
AF = mybir.ActivationFunctionType
ALU = mybir.AluOpType

DIM = 2048
N_HEADS = 16
N_KV_HEADS = 4
HD = 128
EPS = 1e-6
BS = 2
NC_CORES = 8
HPC = N_HEADS // NC_CORES      # q heads per core = 2
ECH = DIM // 128               # e-chunks = 16
TBS = 512                      # token block size (phase 1)
QBS = 512                      # q block size (attention)

DEN_PE = False                 # denominator on PE (else DVE bf16 adds)
PT_DT = F32R if DEN_PE else BF16


def build_program(seq=4096, no_collective=False):
    T = BS * seq
    NTB = T // TBS                 # 16 token blocks
    QB = seq // QBS                # 8 q-blocks per batch
    TPC = T // NC_CORES            # 1024 tokens per core (output slice)
    NT = TPC // 128                # 8
    nch = max(1, QBS // TPC)
    inner = min(QBS, TPC)

    nc = bacc.Bacc("TRN2", target_bir_lowering=False, debug=False,
                   num_devices=NC_CORES)

    xT = nc.dram_tensor("xT", [DIM, T], F32R, kind="ExternalInput").ap()
    wqT = nc.dram_tensor("wqT", [DIM, HPC * HD], F32R,
                           kind="ExternalInput").ap()
    wkT = nc.dram_tensor("wkT", [DIM, HD], F32R, kind="ExternalInput").ap()
    wvT = nc.dram_tensor("wvT", [DIM, HD], F32R, kind="ExternalInput").ap()
    woT = nc.dram_tensor("woT", [DIM, DIM], BF16, kind="ExternalInput").ap()
    ccd = nc.dram_tensor("ccd", [128, seq], BF16, kind="ExternalInput").ap()
    ssd = nc.dram_tensor("ssd", [128, seq], BF16, kind="ExternalInput").ap()
    maskd = nc.dram_tensor("maskd", [2, 128, 2, QBS], BF16,
                           kind="ExternalInput").ap()
    maskhd = nc.dram_tensor("maskhd", [128, 2, QBS // 2], BF16,
                            kind="ExternalInput").ap()
    onesd = nc.dram_tensor("onesd", [128, 128], F32R,
                            kind="ExternalInput").ap()
    identd = nc.dram_tensor("identd", [128, 128], F32R,
                            kind="ExternalInput").ap()
    out = nc.dram_tensor("out", [TPC, DIM], F32, kind="ExternalOutput").ap()

    with tile.TileContext(nc) as tc:
        with (
            tc.tile_pool(name="singles", bufs=1) as singles,
            tc.tile_pool(name="dram", bufs=1, space="DRAM") as dram,
        ):
            # ---- resident SBUF tensors ----
            wq_sb = singles.tile([128, ECH, HPC * HD], F32R)
            nc.scalar.dma_start(out=wq_sb,
                                in_=wqT.rearrange("(ec p) m -> p ec m", p=128))
            wk_sb = singles.tile([128, ECH, HD], F32R)
            nc.scalar.dma_start(out=wk_sb,
                                in_=wkT.rearrange("(ec p) m -> p ec m", p=128))
            wv_sb = singles.tile([128, ECH, HD], F32R)
            nc.scalar.dma_start(out=wv_sb,
                                in_=wvT.rearrange("(ec p) m -> p ec m", p=128))
            ones_sb = singles.tile([128, 128], F32R)
            nc.sync.dma_start(out=ones_sb, in_=onesd)
            onesb_sb = singles.tile([128, 128], BF16)
            nc.scalar.copy(onesb_sb[:, :], ones_sb[:, :])
            id_sb = singles.tile([128, 128], F32R)
            nc.sync.dma_start(out=id_sb, in_=identd)
            cc_sb = singles.tile([128, seq], BF16)
            nc.scalar.dma_start(out=cc_sb, in_=ccd)
            ss_sb = singles.tile([128, seq], BF16)
            nc.scalar.dma_start(out=ss_sb, in_=ssd)
            mask_sb = singles.tile([128, 2, 2, QBS], BF16)
            nc.scalar.dma_start(out=mask_sb,
                                in_=maskd.rearrange("s p c j -> p s c j"))
            maskh_sb = singles.tile([128, 2, QBS // 2], BF16)
            nc.scalar.dma_start(out=maskh_sb, in_=maskhd)
            K_sb = singles.tile([128, T], BF16)          # normed+roped K (d major)
            V_sb = singles.tile([128, T // 128, HD], BF16)  # token-major V
            Q_sb = singles.tile([128, HPC, T], BF16)     # normed+roped Q (d major)
            ebq_sb = singles.tile([128, 1], F32)
            nc.vector.memset(ebq_sb, float(HD) * EPS)
            ebk_sb = singles.tile([128, 1], F32)
            nc.vector.memset(ebk_sb, EPS)

            # two dest-half AllToAll chunks: chunk c carries tokens
            # [d*TPC + c*QBS, +QBS) for every dest core d = exactly the
            # outputs of attention units with qb%2 == c
            a2a_in = [dram.tile([NC_CORES, HPC, HD, QBS], BF16,
                                name=f"a2a_in{c}") for c in range(2)]
            a2a_out = [dram.tile([NC_CORES, HPC, HD, QBS], BF16,
                                 name=f"a2a_out{c}") for c in range(2)]

            # ================= Phase 1: projections =================
            # Per 512-token block: 64 interleaved projection matmuls into a
            # 3-deep rotating PSUM pool, immediate ScalarE drains to SBUF,
            # stats (ones-matmul + Sqrt + DVE recip) and rope on the drained
            # copies. The previous block's PE ops (stat matmuls, V
            # transposes) are spaced into the current block's matmul stream
            # so the PE never idles.
            with (
                tc.tile_pool(name="xt", bufs=3) as xtpool,
                tc.tile_pool(name="pk", bufs=2) as pkpool,
                tc.tile_pool(name="pstat", bufs=1) as pstat,
                tc.tile_pool(name="pstatr", bufs=2) as pstatr,
                tc.tile_pool(name="prope", bufs=2) as prope,
                tc.tile_pool(name="pjps", bufs=3, space="PSUM") as pjps,
                tc.tile_pool(name="statps", bufs=1, space="PSUM") as statps,
                tc.tile_pool(name="vtps", bufs=1, space="PSUM") as vtps,
            ):
                xTr = xT.rearrange("(ec p) t -> p ec t", p=128)

                def emit_proj(tb, extras):
                    """Projection matmuls for block tb; pops one deferred
                    PE-op closure from `extras` every 8 matmuls. Returns
                    (pkq, pkv) SBUF drains."""
                    t0 = tb * TBS
                    qq = pjps.tile([128, 2, TBS], F32, tag="pj",
                                   name=f"qq{tb}")
                    kv = pjps.tile([128, 2, TBS], F32, tag="pj",
                                   name=f"kv{tb}")
                    for half in range(4):
                        xh = xtpool.tile([128, ECH // 4, TBS], F32R, tag="xh")
                        nc.sync.dma_start(
                            out=xh,
                            in_=xTr[:, half * (ECH // 4):(half + 1) * (ECH // 4),
                                    t0:t0 + TBS])
                        for e8 in range(ECH // 4):
                            ec = half * (ECH // 4) + e8
                            st = ec == 0
                            sp = ec == ECH - 1
                            nc.tensor.matmul(qq[:, 0, :], wq_sb[:, ec, 0:128],
                                             xh[:, e8, :], start=st, stop=sp,
                                             skip_group_check=True)
                            nc.tensor.matmul(qq[:, 1, :], wq_sb[:, ec, 128:256],
                                             xh[:, e8, :], start=st, stop=sp,
                                             skip_group_check=True)
                            nc.tensor.matmul(kv[:, 0, :], wk_sb[:, ec, :],
                                             xh[:, e8, :], start=st, stop=sp,
                                             skip_group_check=True)
                            nc.tensor.matmul(kv[:, 1, :], wv_sb[:, ec, :],
                                             xh[:, e8, :], start=st, stop=sp,
                                             skip_group_check=True)
                            if ec % 2 == 1 and extras:
                                extras.pop(0)()
                    pkq = pkpool.tile([128, 2, TBS], F32R, tag="pkq")
                    nc.scalar.copy(pkq[:, :, :], qq[:, :, :])
                    swq = pkpool.tile([128, 2, TBS], F32R, tag="swq")
                    nc.vector.tensor_copy(swq[0:64, :, :], pkq[64:128, :, :])
                    nc.vector.tensor_copy(swq[64:128, :, :], pkq[0:64, :, :])
                    pkv = pkpool.tile([128, 2, TBS], F32R, tag="pkv")
                    nc.scalar.copy(pkv[:, :, :], kv[:, :, :])
                    swk = pkpool.tile([128, TBS], F32R, tag="swk")
                    nc.vector.tensor_copy(swk[0:64, :], pkv[64:128, 0, :])
                    nc.vector.tensor_copy(swk[64:128, :], pkv[0:64, 0, :])
                    return pkq, swq, pkv, swk

                def make_extras(tb, pkq, swq, pkv, swk):
                    """Deferred ops for a drained block: stats, rope, V
                    transpose. PE-bearing closures get spaced into the next
                    block's matmul stream; pure Act/DVE ops ride along."""
                    t0 = tb * TBS
                    s_off = t0 % seq
                    sq = pstat.tile([128, 3, TBS], F32R, tag="sq")
                    sv = pstat.tile([128, 3, TBS], F32, tag="sv")
                    rvv = pstatr.tile([128, 3, TBS], F32, tag="rv")
                    ext = []

                    def sq_q():
                        nc.scalar.activation(sq[:, 0:2, :], pkq[:, :, :],
                                             AF.Square)
                        nc.scalar.activation(sq[:, 2, :], pkv[:, 0, :],
                                             AF.Square,
                                             scale=1.0 / float(np.sqrt(HD)))
                    ext.append(sq_q)

                    def stat(j):
                        def _f():
                            ssb = statps.tile([128, TBS], F32, tag="ss",
                                              name=f"ss{tb}_{j}")
                            nc.tensor.matmul(ssb[:, :], ones_sb[:, :],
                                             sq[:, j, :], start=True,
                                             stop=True, skip_group_check=True)
                            nc.scalar.activation(sv[:, j, :], ssb[:, :],
                                                 AF.Sqrt, bias=ebk_sb[:, :])
                        return _f
                    for j in range(3):
                        ext.append(stat(j))

                    def recip():
                        nc.vector.reciprocal(rvv[:, :, :], sv[:, :, :])
                    ext.append(recip)

                    def rope(src, srcsw, rvs, dst):
                        def _f():
                            cs = cc_sb[:, s_off:s_off + TBS]
                            sn = ss_sb[:, s_off:s_off + TBS]
                            pp = prope.tile([128, TBS], F32, tag="pp")
                            nc.vector.tensor_mul(pp[:, :], src, cs)
                            qq_ = prope.tile([128, TBS], F32, tag="qq_")
                            nc.vector.tensor_mul(qq_[:, :], srcsw, sn)
                            nc.vector.tensor_add(pp[:, :], pp[:, :],
                                                 qq_[:, :])
                            nc.vector.tensor_mul(dst, pp[:, :], rvs)
                        return _f
                    for h in range(HPC):
                        ext.append(rope(pkq[:, h, :], swq[:, h, :],
                                        rvv[:, h, :],
                                        Q_sb[:, h, t0:t0 + TBS]))
                    ext.append(rope(pkv[:, 0, :], swk[:, :], rvv[:, 2, :],
                                    K_sb[:, t0:t0 + TBS]))

                    vt = vtps.tile([128, 4, 128], F32R, tag="vt",
                                   name=f"vt{tb}")

                    def vtr(cch):
                        def _f():
                            nc.tensor.transpose(
                                vt[:, cch, :],
                                pkv[:, 1, cch * 128:(cch + 1) * 128],
                                id_sb[:, :])
                        return _f
                    for cch in range(4):
                        ext.append(vtr(cch))

                    def vcopy():
                        nc.scalar.copy(V_sb[:, tb * 4:(tb + 1) * 4, :],
                                       vt[:, :, :])
                    ext.append(vcopy)
                    return ext

                extras = []
                for tb in range(NTB):
                    drains = emit_proj(tb, extras)
                    extras = extras + make_extras(tb, *drains)
                for e in extras:
                    e()

            def emit_collective(c):
                if no_collective:
                    nc.sync.dma_start(out=a2a_out[c], in_=a2a_in[c])
                else:
                    nc.gpsimd.collective_compute(
                        "AllToAll", ALU.bypass,
                        replica_groups=[list(range(NC_CORES))],
                        ins=[a2a_in[c].opt()], outs=[a2a_out[c].opt()],
                    )

            # ============ Phase 2+3: attention, collectives, wo ============
            with (
                tc.tile_pool(name="apt", bufs=6) as aptpool,
                tc.tile_pool(name="aden", bufs=2) as adenpool,
                tc.tile_pool(name="amisc", bufs=4) as amisc,
                tc.tile_pool(name="wao", bufs=2) as waopool,
                tc.tile_pool(name="wwt", bufs=6) as wwtpool,
                tc.tile_pool(name="wdr", bufs=4) as wdrpool,
            ):
                aps_stack = tc.tile_pool(name="sps", bufs=2, space="PSUM")
                spsps = aps_stack.__enter__()
                aouts_stack = tc.tile_pool(name="outps", bufs=2, space="PSUM")
                outps = aouts_stack.__enter__()
                astat_stack = tc.tile_pool(name="astat", bufs=2, space="PSUM")
                astatps = astat_stack.__enter__()
                ao_sbs = {}
                wt_tiles = {}
                wt_specs = [(c, eb, hc) for c in range(2)
                            for eb in range(DIM // 512) for hc in range(ECH)]

                def load_aosb(c):
                    ao_sb = waopool.tile([128, ECH, QBS], BF16, tag="aosb",
                                         name=f"aosb{c}")
                    aor = a2a_out[c].rearrange("r h p t -> p (r h) t")
                    for hc in range(ECH):
                        nc.gpsimd.dma_start(out=ao_sb[:, hc, :],
                                            in_=aor[:, hc, :])
                    ao_sbs[c] = ao_sb

                def load_wt(spec):
                    c_, eb, hc = spec
                    wt = wwtpool.tile([128, 512], BF16, tag="wt")
                    nc.sync.dma_start(
                        out=wt,
                        in_=woT[hc * 128:(hc + 1) * 128,
                                eb * 512:(eb + 1) * 512])
                    wt_tiles[spec] = wt
                # software-pipelined: PV/den matmuls for probability tile g
                # are emitted after the NEXT group's score matmuls, so the PE
                # always has exp-independent work while ScalarE runs exp.
                pending = []   # deferred (pt, o_ps, den_ps, b, g, ng) PV/den

                def flush_pv():
                    while pending:
                        pt, o_ps, den_ps, b_, g_, ng_ = pending.pop(0)
                        last = g_ == ng_ - 1
                        qw = QBS // 2 if last else QBS
                        q0 = QBS - qw
                        for ci in range(2):
                            kcol = b_ * seq + g_ * 256 + ci * 128
                            nc.tensor.matmul(
                                o_ps[:, q0:QBS],
                                V_sb[:, kcol // 128, :],
                                pt[:, ci, 0:qw],
                                start=(g_ == 0 and ci == 0),
                                stop=(last and ci == 1),
                                skip_group_check=True)

                finish = []    # deferred per-unit normalize+store closures

                def flush_finish():
                    while finish:
                        finish.pop(0)()

                unit_order = [(b, qb) for par in (0, 1)
                              for b in range(BS)
                              for qb in range(par, QB, 2)]
                n_even = sum(1 for b, qb in unit_order if qb % 2 == 0)
                for ui, (b, qb) in enumerate(unit_order):
                    if True:
                        tok0 = b * seq + qb * QBS
                        ng = 2 * (qb + 1)
                        for h in range(HPC):
                            o_ps = outps.tile([128, QBS], F32, tag="ops")
                            den_ps = astatps.tile([128, QBS], F32, tag="denp")
                            den_sb = None
                            if not DEN_PE:
                                den_sb = adenpool.tile([128, QBS], BF16,
                                                       tag="den")
                            qmv = Q_sb[:, h, tok0:tok0 + QBS]
                            for g in range(ng):
                                last = g == ng - 1
                                qw = QBS // 2 if last else QBS
                                q0 = QBS - qw
                                sps = spsps.tile([128, 2, QBS], F32, tag="sps")
                                for ci in range(2):
                                    kcol = b * seq + g * 256 + ci * 128
                                    nc.tensor.matmul(
                                        sps[:, ci, 0:qw],
                                        K_sb[:, kcol:kcol + 128],
                                        Q_sb[:, h, tok0 + q0:tok0 + QBS],
                                        start=True, stop=True,
                                        skip_group_check=True)
                                flush_pv()
                                flush_finish()
                                pt = aptpool.tile([128, 2, QBS], PT_DT, tag="pt")
                                nc.scalar.activation(pt[:, :, 0:qw],
                                                     sps[:, :, 0:qw], AF.Exp)
                                if last:
                                    nc.vector.tensor_mul(
                                        pt[:, :, 0:qw], pt[:, :, 0:qw],
                                        maskh_sb[:, :, :])
                                elif g == ng - 2:
                                    nc.vector.tensor_mul(
                                        pt[:, :, :], pt[:, :, :],
                                        mask_sb[:, 0, :, :])
                                pending.append((pt, o_ps, den_ps, b, g, ng))
                                if not DEN_PE:
                                    if g == 0:
                                        nc.vector.tensor_add(den_sb[:, :],
                                                             pt[:, 0, :],
                                                             pt[:, 1, :])
                                    else:
                                        tmp = amisc.tile([128, QBS], BF16,
                                                         tag="dtmp")
                                        nc.vector.tensor_add(
                                            tmp[:, 0:qw],
                                            pt[:, 0, 0:qw],
                                            pt[:, 1, 0:qw])
                                        nc.vector.scalar_tensor_tensor(
                                            den_sb[:, q0:QBS],
                                            tmp[:, 0:qw], 1.0,
                                            den_sb[:, q0:QBS],
                                            ALU.mult, ALU.add)

                            def unit_finish(o_ps=o_ps, den_ps=den_ps,
                                            den_sb=den_sb, tok0=tok0, h=h):
                                if not DEN_PE:
                                    nc.tensor.matmul(den_ps[:, :],
                                                     onesb_sb[:, :],
                                                     den_sb[:, :],
                                                     start=True, stop=True,
                                                     skip_group_check=True)
                                rv = amisc.tile([128, QBS], F32, tag="arv")
                                nc.vector.reciprocal(rv[:, :], den_ps[:, :])
                                ao = amisc.tile([128, QBS], BF16, tag="ao")
                                nc.vector.tensor_mul(ao[:, :], o_ps[:, :],
                                                     rv[:, :])
                                d = tok0 // TPC
                                nc.sync.dma_start(
                                    out=a2a_in[qb % 2][d, h, :, :],
                                    in_=ao[:, :])

                            finish.append(unit_finish)
                    if ui == n_even - 1:
                        flush_pv()
                        flush_finish()
                        emit_collective(0)
                        load_aosb(0)
                        for spec in wt_specs[:6]:
                            load_wt(spec)
                flush_pv()
                flush_finish()
                emit_collective(1)
                load_aosb(1)

                # close attention PSUM pools so wo reuses the banks
                astat_stack.__exit__(None, None, None)
                aouts_stack.__exit__(None, None, None)
                aps_stack.__exit__(None, None, None)
                # ---- wo projection (same SBUF pool scope) ----
                with tc.tile_pool(name="wops", bufs=2 * (NT // 2),
                                  space="PSUM") as wops:
                    NTH = NT // 2          # token tiles per half-pass
                    nspec = 6
                    for c in range(2):
                        ao_sb = ao_sbs[c]
                        for eb in range(DIM // 512):
                            ops = [wops.tile([128, 512], F32, tag="wps",
                                             name=f"wps{c}_{eb}_{i}")
                                   for i in range(NTH)]
                            for hc in range(ECH):
                                spec = (c, eb, hc)
                                if spec not in wt_tiles:
                                    load_wt(spec)
                                wt = wt_tiles[spec]
                                for tt in range(NTH):
                                    nc.tensor.matmul(
                                        ops[tt][:, :],
                                        ao_sb[:, hc, tt * 128:(tt + 1) * 128],
                                        wt[:, :],
                                        start=(hc == 0), stop=(hc == ECH - 1),
                                        skip_group_check=True)
                            for tt in range(NTH):
                                od = wdrpool.tile([128, 512], F32, tag="od")
                                if tt % 2 == 0:
                                    nc.scalar.copy(od[:, :], ops[tt][:, :])
                                else:
                                    nc.vector.tensor_copy(od[:, :],
                                                          ops[tt][:, :])
                                row = c * QBS + tt * 128
                                nc.gpsimd.dma_start(
                                    out=out[row:row + 128,
                                            eb * 512:(eb + 1) * 512],
                                    in_=od)
    nc.compile()
    return nc


# ---------------- host-side prep / run ----------------

_PROG_CACHE = {}


def _get_program(seq):
    if seq not in _PROG_CACHE:
        _PROG_CACHE[seq] = build_program(seq)
    return _PROG_CACHE[seq]


def _rot_perm():
    return np.concatenate([np.arange(0, HD, 2), np.arange(1, HD, 2)])


def make_inputs(x, freqs_cis, wq, wk, wv, wo, q_norm_w, k_norm_w):
    bs, seq, _ = x.shape
    T = bs * seq
    perm = _rot_perm()

    xT = np.ascontiguousarray(x.reshape(T, DIM).T.astype(np.float32))
    woT = np.ascontiguousarray(wo.T.astype(ml_dtypes.bfloat16))
    cos = freqs_cis[:, :, 0].T.astype(np.float32)   # [64, seq]
    sin = freqs_cis[:, :, 1].T.astype(np.float32)
    ccd = np.ascontiguousarray(
        np.concatenate([cos, cos], axis=0).astype(ml_dtypes.bfloat16))
    ssd = np.ascontiguousarray(
        np.concatenate([-sin, sin], axis=0).astype(ml_dtypes.bfloat16))

    # masks[0]: second-to-last 256-key group vs all 512 q (full tile)
    # masks[1]: last 256-key group vs q in [256, 512) only (half tile)
    masks = np.zeros((2, 128, 2, QBS), dtype=ml_dtypes.bfloat16)
    for s in range(2):
        for c in range(2):
            k_rel = s * 256 + c * 128 + np.arange(128)[:, None]
            masks[s, :, c, :] = (k_rel <= np.arange(QBS)[None, :]).astype(
                ml_dtypes.bfloat16)
    maskh = np.ascontiguousarray(masks[1, :, :, 256:])

    onesd = np.ones((128, 128), dtype=np.float32)
    identd = np.eye(128, dtype=np.float32)

    in_maps = []
    for c in range(NC_CORES):
        g = c // 2
        wq_rows = wq[c * HPC * HD:(c + 1) * HPC * HD].reshape(HPC, HD, DIM)
        wq_rows = wq_rows[:, perm, :].reshape(HPC * HD, DIM)
        wk_rows = wk[g * HD:(g + 1) * HD][perm]
        wv_rows = wv[g * HD:(g + 1) * HD]
        in_maps.append({
            "xT": xT,
            "wqT": np.ascontiguousarray(wq_rows.T.astype(np.float32)),
            "wkT": np.ascontiguousarray(wk_rows.T.astype(np.float32)),
            "wvT": np.ascontiguousarray(wv_rows.T.astype(np.float32)),
            "woT": woT,
            "ccd": ccd,
            "ssd": ssd,
            "maskd": masks,
            "maskhd": maskh,
            "onesd": onesd,
            "identd": identd,
        })
    return in_maps


def run(x, freqs_cis, wq, wk, wv, wo, q_norm_w, k_norm_w, trace=False):
    bs, seq, _ = x.shape
    nc = _get_program(seq)
    in_maps = make_inputs(x, freqs_cis, wq, wk, wv, wo, q_norm_w, k_norm_w)
    res = None
    for attempt in range(3):
        try:
            res = run_bass_kernel_spmd(nc, in_maps, list(range(NC_CORES)),
                                       trace=trace)
            break
        except Exception:
            if attempt == 2:
                raise
    shards = [res.results[c]["out"] for c in range(NC_CORES)]
    full = np.concatenate(shards, axis=0).reshape(bs, seq, DIM)
    return full, res


def kernel(x, freqs_cis, wq, wk, wv, wo, q_norm_w, k_norm_w):
    out, _ = run(np.asarray(x, np.float32), np.asarray(freqs_cis, np.float32),
                 np.asarray(wq, np.float32), np.asarray(wk, np.float32),
                 np.asarray(wv, np.float32), np.asarray(wo, np.float32),
                 np.asarray(q_norm_w, np.float32), np.asarray(k_norm_w, np.float32))
    return out

